# revision 15
# baseline (speedup 1.0000x reference)
"""CrossNetMix (DCN-V2 mixture-of-low-rank-experts) Trainium2 kernel.

Strategy: data-parallel over batch across 8 cores (2048 rows each), with all
tensors kept feature-major on chip ([d, b] layout) so the contraction dim of
every matmul lands on SBUF partitions and no on-chip transposes are needed.
The host pre-shuffles every tensor into the exact [partition, ...] layout the
kernel reads, so each DMA moves long contiguous per-partition runs.

Matmul operands are bf16 (fp32 PSUM accumulation), except the U stage which
runs fp8e4 DoubleRow (2 weights per PE cell, half the matmuls): the gate
weights are pre-scaled so y = h2*(4w) and U8 = 16*U land comfortably in
e4m3 range, making ups = 64x the true value; the combine consumes a
pre-divided x0/64 copy so no descale op is needed. The loop is layer-major
over all 4 batch chunks so chunk c+1's matmuls fill chunk c's end-of-layer
dependency gap and the PE never idles long enough to re-throttle.

Per layer (fused), per chunk:
  g = xi @ Wg.T                  -> gating matmuls, M=8
  w = softmax(g)                 -> exp on ACT; partition-sum / broadcast via
                                    tiny ones/selector matmuls on the PE
  h1 = tanh(xi @ Vflat)          -> [er=512, b] feature-major
  h2 = tanh(blockdiag_C @ h1)    -> per-expert C folded into 128x128
                                    block-diagonal pairs (4 matmuls)
  y  = h2 * 4w_broadcast (fp8)
  ups = U8.T @ y  (DoubleRow)    -> [d, b], 64x scaled
  xi' = (x0/64) * (ups + 64b) + xi -> fused combine from PSUM into fresh tile
"""

import sys

import numpy as np

if "/opt/trn_rl_repo" not in sys.path:
    sys.path.insert(0, "/opt/trn_rl_repo")

import concourse.bass as bass
import concourse.bacc as bacc
import concourse.mybir as mybir
from concourse.tile import TileContext
from concourse.bass_utils import run_bass_kernel_spmd

import ml_dtypes

AF = mybir.ActivationFunctionType
OP = mybir.AluOpType
PM = mybir.MatmulPerfMode
F32 = mybir.dt.float32
BF16 = mybir.dt.bfloat16
FP8 = mybir.dt.float8e4
NPBF = ml_dtypes.bfloat16
NPF8 = ml_dtypes.float8_e4m3

N_CROSS = 3
E = 8            # experts
D = 1024         # feature dim
R = 64           # low rank
B = 16384        # full batch
NCORES = 8
BC = B // NCORES  # rows per core
CHUNK = 512       # batch tile (matmul free dim)
NCHUNK = BC // CHUNK
P = 128
KC = D // P       # d-chunks
ER = E * R        # 512
MC = ER // P      # (e,r)-chunks

USCALE = 16.0     # host-side U-weight scale (fp8 range centering)
YSCALE = 4.0      # gate-weight scale applied on-chip via the recip row
VSCALE = 16.0     # host-side V-weight scale (descaled inside the h1 tanh)


def _build():
    nc = bacc.Bacc(None)
    # every DRAM tensor is laid out partition-major on the host so DMAs
    # stream long contiguous runs per partition
    xq = nc.declare_dram_parameter("xq", [NCHUNK, P, KC, CHUNK], BF16, isOutput=False)
    xqd = nc.declare_dram_parameter("xqd", [NCHUNK, P, KC, CHUNK], BF16, isOutput=False)
    xq8 = nc.declare_dram_parameter("xq8", [NCHUNK, P, KC, CHUNK], FP8, isOutput=False)
    V8p = nc.declare_dram_parameter("V8p", [P, N_CROSS, MC, KC, P], FP8, isOutput=False)
    Cp = nc.declare_dram_parameter("Cp", [P, N_CROSS, MC, P], BF16, isOutput=False)
    U8p = nc.declare_dram_parameter("U8p", [P, N_CROSS, MC, D], FP8, isOutput=False)
    Wgp = nc.declare_dram_parameter("Wgp", [P, KC, E], BF16, isOutput=False)
    bp = nc.declare_dram_parameter("bp", [P, N_CROSS, KC], F32, isOutput=False)
    selp = nc.declare_dram_parameter("selp", [E, MC + 1, P], BF16, isOutput=False)
    outq = nc.declare_dram_parameter("outq", [NCHUNK, P, KC, CHUNK], BF16, isOutput=True)

    with TileContext(nc) as tc:
        with (
            tc.sbuf_pool(name="wpool", bufs=1) as wpool,
            tc.sbuf_pool(name="xpool", bufs=NCHUNK) as xpool,
            tc.sbuf_pool(name="xdpool", bufs=NCHUNK) as xdpool,
            tc.sbuf_pool(name="x8pool", bufs=6) as x8pool,
            tc.sbuf_pool(name="xipool", bufs=6) as xipool,
            tc.sbuf_pool(name="h1pool", bufs=6) as h1pool,
            tc.sbuf_pool(name="h2pool", bufs=3) as h2pool,
            tc.sbuf_pool(name="ypool", bufs=3) as ypool,
            tc.sbuf_pool(name="tpool", bufs=4) as tpool,
            tc.sbuf_pool(name="spool", bufs=2) as spool,
            tc.psum_pool(name="psmm", bufs=3) as psmm,
            tc.psum_pool(name="psu", bufs=2) as psu,
            tc.psum_pool(name="pswb", bufs=2) as pswb,
            tc.psum_pool(name="psg", bufs=1) as psg,
        ):
            v_sb = wpool.tile([P, N_CROSS, MC, KC, P], FP8)
            u_sb = wpool.tile([P, N_CROSS, MC, D], FP8)
            c_sb = wpool.tile([P, N_CROSS, MC, P], BF16)

            def load_x0(c, parts=2, pool=xpool, src_dram=xq, tag="x0"):
                t = pool.tile([P, KC, CHUNK], BF16, tag=tag, name=f"{tag}_{c}")
                step = KC // parts
                for q in range(parts):
                    sl = slice(q * step, (q + 1) * step)
                    nc.sync.dma_start(t[:, sl], src_dram[c, :, sl])
                return t

            # critical-path-first DMA order: gate weights, then interleaved
            # chunk-0 x parts with layer-0 V parts; then everything else.
            def load_x8(c):
                t = x8pool.tile([P, KC, CHUNK], FP8, tag="x8", name=f"x8_{c}")
                nc.sync.dma_start(t, xq8[c])
                return t

            wg_sb = wpool.tile([P, KC, E], BF16)
            nc.sync.dma_start(wg_sb, Wgp[:])
            x0_t = xpool.tile([P, KC, CHUNK], BF16, tag="x0", name="x0_0")
            x8_t = x8pool.tile([P, KC, CHUNK], FP8, tag="x8", name="x8_0")
            for q in range(4):
                sl = slice(q * 2, (q + 1) * 2)
                nc.sync.dma_start(x8_t[:, sl], xq8[0, :, sl])
                nc.sync.dma_start(v_sb[:, 0, q], V8p[:, 0, q])
                nc.sync.dma_start(x0_t[:, sl], xq[0, :, sl])
            x0_tiles = [x0_t]
            x8_tiles = {(0, 0): x8_t}
            sel_sb = wpool.tile([E, MC + 1, P], BF16)
            nc.sync.dma_start(sel_sb, selp[:])
            b_sb = wpool.tile([P, N_CROSS, KC], F32)
            nc.sync.dma_start(b_sb, bp[:])
            nc.sync.dma_start(c_sb[:, 0], Cp[:, 0])
            nc.sync.dma_start(u_sb[:, 0], U8p[:, 0])
            x0d_tiles = [load_x0(0, pool=xdpool, src_dram=xqd, tag="x0d")]
            for c in range(1, NCHUNK):
                x8_tiles[(c, 0)] = load_x8(c)
                x0_tiles.append(load_x0(c))
                x0d_tiles.append(load_x0(c, pool=xdpool, src_dram=xqd, tag="x0d"))
            for i in range(1, N_CROSS):
                for mc in range(MC):
                    nc.sync.dma_start(v_sb[:, i, mc], V8p[:, i, mc])
                nc.sync.dma_start(c_sb[:, i], Cp[:, i])
                nc.sync.dma_start(u_sb[:, i], U8p[:, i])

            xi_tiles = {(c, 0): x0_tiles[c] for c in range(NCHUNK)}

            for i in range(N_CROSS):
                for c in range(NCHUNK):
                    x0d = x0d_tiles[c]
                    src = xi_tiles.pop((c, i))
                    src8 = x8_tiles.pop((c, i))
                    xi = xipool.tile([P, KC, CHUNK], BF16, tag="xi")
                    xi_tiles[(c, i + 1)] = xi
                    if i + 1 < N_CROSS:
                        xi8 = x8pool.tile([P, KC, CHUNK], FP8, tag="x8")
                        x8_tiles[(c, i + 1)] = xi8
                    # ---- gating: g[e, b], then w = softmax over e ----
                    gps = psg.tile([E, CHUNK], F32, tag="g")
                    for kc in range(KC):
                        nc.tensor.matmul(
                            gps,
                            wg_sb[:, kc, :],
                            src[:, kc, :],
                            start=(kc == 0),
                            stop=(kc == KC - 1),
                        )
                    expg = spool.tile([E, CHUNK], BF16, tag="expg")
                    nc.scalar.activation(expg, gps, AF.Exp)
                    sums = psg.tile([1, CHUNK], F32, tag="g")
                    nc.tensor.matmul(
                        sums, sel_sb[:, MC, 0:1], expg, start=True, stop=True
                    )
                    rfast = spool.tile([1, CHUNK], F32, tag="rfast")
                    nc.vector.reciprocal_approx_fast(rfast, sums)
                    rrow = spool.tile([1, CHUNK], BF16, tag="rrow")
                    nc.vector.tensor_scalar_mul(rrow, rfast, YSCALE)
                    wps = psg.tile([E, CHUNK], F32, tag="g")
                    nc.tensor.matmul(
                        wps, sel_sb[0:1, MC, 0:E], rrow, start=True, stop=True
                    )
                    wsb = spool.tile([E, CHUNK], BF16, tag="wsb")
                    nc.vector.tensor_tensor(wsb, expg, wps, OP.mult)
                    # ---- V stage: h1 = tanh(V8flat.T @ xi8 / VSCALE) ----
                    h1s = []
                    for mc in range(MC):
                        vps = psmm.tile([P, CHUNK], F32, tag="mm")
                        for kk in range(KC // 2):
                            nc.tensor.matmul(
                                vps,
                                v_sb[:, i, mc, 2 * kk : 2 * kk + 2, :],
                                src8[:, 2 * kk : 2 * kk + 2, :],
                                start=(kk == 0),
                                stop=(kk == KC // 2 - 1),
                                perf_mode=PM.DoubleRow,
                            )
                        h1 = h1pool.tile([P, CHUNK], BF16, tag="h1")
                        nc.scalar.activation(h1, vps, AF.Tanh, scale=1.0 / VSCALE)
                        h1s.append(h1)
                    # ---- C stage (block-diag expert pairs) + gate scale ----
                    y8 = ypool.tile([P, MC, CHUNK], FP8, tag="y")
                    for mc in range(MC):
                        cps = psmm.tile([P, CHUNK], F32, tag="mm")
                        nc.tensor.matmul(
                            cps, c_sb[:, i, mc, :], h1s[mc], start=True, stop=True
                        )
                        wbp = pswb.tile([P, CHUNK], F32, tag="wb")
                        nc.tensor.matmul(
                            wbp, sel_sb[:, mc, :], wsb, start=True, stop=True
                        )
                        h2 = h2pool.tile([P, CHUNK], BF16, tag="h2")
                        nc.scalar.activation(h2, cps, AF.Tanh)
                        nc.vector.tensor_tensor(y8[:, mc, :], h2, wbp, OP.mult)
                    # ---- U stage (fp8 DoubleRow) + fused combine ----
                    for dc in range(KC):
                        ups = psu.tile([P, CHUNK], F32, tag="u")
                        for m2 in range(MC // 2):
                            nc.tensor.matmul(
                                ups,
                                u_sb[:, i, 2 * m2 : 2 * m2 + 2, dc * P : (dc + 1) * P],
                                y8[:, 2 * m2 : 2 * m2 + 2, :],
                                start=(m2 == 0),
                                stop=(m2 == MC // 2 - 1),
                                perf_mode=PM.DoubleRow,
                            )
                        tmp = tpool.tile([P, CHUNK], BF16, tag="tmp")
                        nc.vector.scalar_tensor_tensor(
                            tmp,
                            ups,
                            b_sb[:, i, dc : dc + 1],
                            x0d[:, dc, :],
                            OP.add,
                            OP.mult,
                        )
                        eng = nc.gpsimd if dc % 2 == 0 else nc.vector
                        eng.tensor_tensor(
                            xi[:, dc, :], tmp, src[:, dc, :], OP.add
                        )
                        if i + 1 < N_CROSS:
                            nc.gpsimd.tensor_copy(xi8[:, dc, :], xi[:, dc, :])
                        else:
                            nc.scalar.dma_start(outq[c, :, dc], xi[:, dc, :])
    nc.compile()
    return nc


_CTX = {}


def _get_nc():
    if "nc" not in _CTX:
        _CTX["nc"] = _build()
    return _CTX["nc"]


def _prep_weights(U, V, C, Wg, b):
    f = np.float32
    U = np.asarray(U, dtype=f)
    V = np.asarray(V, dtype=f)
    C = np.asarray(C, dtype=f)
    Wg = np.asarray(Wg, dtype=f)
    b = np.asarray(b, dtype=f)
    # Vl[i, d, e*R+r] = V[i, e, d, r]; then partition-major
    # V8p[p, i, mc, kc, q] = VSCALE*Vl[i, kc*P+p, mc*P+q]
    Vl = V.transpose(0, 2, 1, 3).reshape(N_CROSS, KC, P, MC, P)
    V8p = np.ascontiguousarray((Vl.transpose(2, 0, 3, 1, 4) * VSCALE).astype(NPF8))
    # Ul[i, e*R+r, d] = U[i, e, d, r]; U8p[p, i, mc, d] = USCALE*Ul[i, mc*P+p, d]
    Ul = U.transpose(0, 1, 3, 2).reshape(N_CROSS, MC, P, D)
    U8p = np.ascontiguousarray((Ul.transpose(2, 0, 1, 3) * USCALE).astype(NPF8))
    # block-diagonal expert pairs for the C stage: Cp[p, i, m, q]
    Cb = np.zeros((N_CROSS, MC, P, P), dtype=f)
    for i in range(N_CROSS):
        for m in range(MC):
            Cb[i, m, :R, :R] = C[i, 2 * m]
            Cb[i, m, R:, R:] = C[i, 2 * m + 1]
    Cp = np.ascontiguousarray(Cb.transpose(2, 0, 1, 3).astype(NPBF))
    # Wgp[p, kc, e] = Wg[e, kc*P+p]
    Wgp = np.ascontiguousarray(
        Wg.reshape(E, KC, P).transpose(2, 1, 0).astype(NPBF)
    )
    # bp[p, i, kc] = (USCALE*YSCALE) * b[i, kc*P+p]  (matches the scaled ups)
    bpa = np.ascontiguousarray(
        b.reshape(N_CROSS, KC, P).transpose(2, 0, 1) * (USCALE * YSCALE)
    )
    # selector planes for broadcasting gate weights over ranks + a ones plane
    sel = np.zeros((E, MC + 1, P), dtype=NPBF)
    for m in range(MC):
        for j in range(P):
            sel[2 * m + j // R, m, j] = 1.0
    sel[:, MC, :] = 1.0
    return dict(V8p=V8p, U8p=U8p, Cp=Cp, Wgp=Wgp, bp=bpa, selp=sel)


def kernel(x, U, V, C, Wg, b, _trace=False):
    nc = _get_nc()
    w = _prep_weights(U, V, C, Wg, b)
    xf = np.asarray(x, dtype=np.float32)
    # xq[c, p, kc, j] = x[core*BC + c*CHUNK + j, kc*P + p]
    xs = xf.reshape(NCORES, NCHUNK, CHUNK, KC, P).transpose(0, 1, 4, 3, 2)
    in_maps = []
    for ci in range(NCORES):
        m = {
            "xq": np.ascontiguousarray(xs[ci].astype(NPBF)),
            "xqd": np.ascontiguousarray(
                (xs[ci] * (1.0 / (USCALE * YSCALE))).astype(NPBF)
            ),
            "xq8": np.ascontiguousarray(xs[ci].astype(NPF8)),
        }
        m.update(w)
        in_maps.append(m)
    res = run_bass_kernel_spmd(nc, in_maps, list(range(NCORES)), trace=_trace)
    kernel.last_result = res
    # outq[c, p, kc, j] -> out[core rows, d]
    outs = []
    for ci in range(NCORES):
        oq = np.asarray(res.results[ci]["outq"])
        outs.append(oq.transpose(0, 3, 2, 1).reshape(BC, D))
    out = np.concatenate(outs, axis=0)
    return np.ascontiguousarray(out.astype(np.float32))


# revision 16
# speedup vs baseline: 1.2871x; 1.2871x over previous
"""CrossNetMix (DCN-V2 mixture-of-low-rank-experts) Trainium2 kernel.

Strategy: data-parallel over batch across 8 cores (2048 rows each), with all
tensors kept feature-major on chip ([d, b] layout) so the contraction dim of
every matmul lands on SBUF partitions and no on-chip transposes are needed.
The host pre-shuffles every tensor into the exact [partition, ...] layout the
kernel reads, so each DMA moves long contiguous per-partition runs.

The V and U stages run fp8e4 DoubleRow matmuls (2 weights per PE cell, half
the matmul count); C / gating stay bf16. Scales keep everything in e4m3
range: V8 = 16*V (descaled inside the h1 tanh), U8 = 16*U and y = h2*(4w)
so ups = 64x the true value; the combine consumes a pre-divided x0/64 copy
so no descale op is needed. The fp8 copy of xi for the next layer's V stage
is produced by ScalarE Copy ops (GpSimd fp8 casts are pathologically slow).

The loop is layer-major over all 4 batch chunks so chunk c+1's matmuls fill
chunk c's end-of-layer dependency gap and the PE never idles long enough to
re-throttle. When b == 0 (the common case) the combine runs on dc-PAIRS
([128, 1024] DVE ops over two PSUM banks) to halve elementwise op count.

Per layer (fused), per chunk:
  g = xi @ Wg.T                  -> gating matmuls, M=8
  w = softmax(g)                 -> exp on ACT; partition-sum / broadcast via
                                    tiny ones/selector matmuls on the PE
  h1 = tanh(xi8 @ V8flat / 16)   -> [er=512, b] fp8 DoubleRow
  h2 = tanh(blockdiag_C @ h1)    -> per-expert C folded into 128x128
                                    block-diagonal pairs (4 matmuls)
  y8 = h2 * 4w_broadcast (fp8)
  ups = U8.T @ y8 (DoubleRow)    -> [d, b], 64x scaled
  xi' = (x0/64) * (ups + 64b) + xi -> fused combine from PSUM into fresh tile
"""

import sys

import numpy as np

if "/opt/trn_rl_repo" not in sys.path:
    sys.path.insert(0, "/opt/trn_rl_repo")

import concourse.bass as bass
import concourse.bacc as bacc
import concourse.mybir as mybir
from concourse.tile import TileContext
from concourse.bass_utils import run_bass_kernel_spmd

import ml_dtypes

AF = mybir.ActivationFunctionType
OP = mybir.AluOpType
PM = mybir.MatmulPerfMode
F32 = mybir.dt.float32
BF16 = mybir.dt.bfloat16
FP8 = mybir.dt.float8e4
NPBF = ml_dtypes.bfloat16
NPF8 = ml_dtypes.float8_e4m3

N_CROSS = 3
E = 8            # experts
D = 1024         # feature dim
R = 64           # low rank
B = 16384        # full batch
NCORES = 8
BC = B // NCORES  # rows per core
CHUNK = 512       # batch tile (matmul free dim)
NCHUNK = BC // CHUNK
P = 128
KC = D // P       # d-chunks
ER = E * R        # 512
MC = ER // P      # (e,r)-chunks

USCALE = 16.0     # host-side U-weight scale (fp8 range centering)
YSCALE = 4.0      # gate-weight scale applied on-chip via the recip row
VSCALE = 16.0     # host-side V-weight scale (descaled inside the h1 tanh)


def _build(bias_zero):
    nc = bacc.Bacc(None)
    # every DRAM tensor is laid out partition-major on the host so DMAs
    # stream long contiguous runs per partition
    xq = nc.declare_dram_parameter("xq", [NCHUNK, P, KC, CHUNK], BF16, isOutput=False)
    xqd = nc.declare_dram_parameter("xqd", [NCHUNK, P, KC, CHUNK], BF16, isOutput=False)
    xq8 = nc.declare_dram_parameter("xq8", [NCHUNK, P, KC, CHUNK], FP8, isOutput=False)
    V8p = nc.declare_dram_parameter("V8p", [P, N_CROSS, MC, KC, P], FP8, isOutput=False)
    Cp = nc.declare_dram_parameter("Cp", [P, N_CROSS, MC, P], BF16, isOutput=False)
    U8p = nc.declare_dram_parameter("U8p", [P, N_CROSS, MC, D], FP8, isOutput=False)
    Wgp = nc.declare_dram_parameter("Wgp", [P, KC, E], BF16, isOutput=False)
    bp = nc.declare_dram_parameter("bp", [P, N_CROSS, KC], F32, isOutput=False)
    selp = nc.declare_dram_parameter("selp", [E, MC + 1, P], BF16, isOutput=False)
    outq = nc.declare_dram_parameter("outq", [NCHUNK, P, KC, CHUNK], BF16, isOutput=True)

    with TileContext(nc) as tc:
        with (
            tc.sbuf_pool(name="wpool", bufs=1) as wpool,
            tc.sbuf_pool(name="xpool", bufs=NCHUNK) as xpool,
            tc.sbuf_pool(name="xdpool", bufs=NCHUNK) as xdpool,
            tc.sbuf_pool(name="x8pool", bufs=6) as x8pool,
            tc.sbuf_pool(name="xipool", bufs=6) as xipool,
            tc.sbuf_pool(name="h1pool", bufs=6) as h1pool,
            tc.sbuf_pool(name="h2pool", bufs=3) as h2pool,
            tc.sbuf_pool(name="ypool", bufs=3) as ypool,
            tc.sbuf_pool(name="tpool", bufs=4) as tpool,
            tc.sbuf_pool(name="spool", bufs=2) as spool,
            tc.psum_pool(name="psmm", bufs=2) as psmm,
            tc.psum_pool(name="psu", bufs=2) as psu,
            tc.psum_pool(name="pswb", bufs=1) as pswb,
            tc.psum_pool(name="psg", bufs=1) as psg,
        ):
            v_sb = wpool.tile([P, N_CROSS, MC, KC, P], FP8)
            u_sb = wpool.tile([P, N_CROSS, MC, D], FP8)
            c_sb = wpool.tile([P, N_CROSS, MC, P], BF16)

            def load_x0(c, parts=2, pool=None, src_dram=None, tag="x0"):
                t = pool.tile([P, KC, CHUNK], BF16, tag=tag, name=f"{tag}_{c}")
                step = KC // parts
                for q in range(parts):
                    sl = slice(q * step, (q + 1) * step)
                    nc.sync.dma_start(t[:, sl], src_dram[c, :, sl])
                return t

            def load_x8(c):
                t = x8pool.tile([P, KC, CHUNK], FP8, tag="x8", name=f"x8_{c}")
                nc.sync.dma_start(t, xq8[c])
                return t

            # critical-path-first DMA order: gate weights, then interleaved
            # chunk-0 x parts with layer-0 V parts; then everything else.
            wg_sb = wpool.tile([P, KC, E], BF16)
            nc.sync.dma_start(wg_sb, Wgp[:])
            ones4 = wpool.tile([1, E], F32)
            nc.vector.memset(ones4, YSCALE)
            x0_t = xpool.tile([P, KC, CHUNK], BF16, tag="x0", name="x0_0")
            x8_t = x8pool.tile([P, KC, CHUNK], FP8, tag="x8", name="x8_0")
            for q in range(4):
                sl = slice(q * 2, (q + 1) * 2)
                nc.sync.dma_start(x8_t[:, sl], xq8[0, :, sl])
                nc.sync.dma_start(v_sb[:, 0, q], V8p[:, 0, q])
                nc.sync.dma_start(x0_t[:, sl], xq[0, :, sl])
            x0_tiles = [x0_t]
            x8_tiles = {(0, 0): x8_t}
            sel_sb = wpool.tile([E, MC + 1, P], BF16)
            nc.sync.dma_start(sel_sb, selp[:])
            b_sb = wpool.tile([P, N_CROSS, KC], F32)
            nc.sync.dma_start(b_sb, bp[:])
            nc.sync.dma_start(c_sb[:, 0], Cp[:, 0])
            nc.sync.dma_start(u_sb[:, 0], U8p[:, 0])
            x0d_tiles = [load_x0(0, pool=xdpool, src_dram=xqd, tag="x0d")]
            for c in range(1, NCHUNK):
                x8_tiles[(c, 0)] = load_x8(c)
                x0_tiles.append(load_x0(c, pool=xpool, src_dram=xq))
                x0d_tiles.append(load_x0(c, pool=xdpool, src_dram=xqd, tag="x0d"))
            for i in range(1, N_CROSS):
                for mc in range(MC):
                    nc.sync.dma_start(v_sb[:, i, mc], V8p[:, i, mc])
                nc.sync.dma_start(c_sb[:, i], Cp[:, i])
                nc.sync.dma_start(u_sb[:, i], U8p[:, i])

            xi_tiles = {(c, 0): x0_tiles[c] for c in range(NCHUNK)}

            for i in range(N_CROSS):
                for c in range(NCHUNK):
                    x0d = x0d_tiles[c]
                    src = xi_tiles.pop((c, i))
                    src8 = x8_tiles.pop((c, i))
                    xi = xipool.tile([P, KC, CHUNK], BF16, tag="xi")
                    xi_tiles[(c, i + 1)] = xi
                    last = i + 1 == N_CROSS
                    if not last:
                        xi8 = x8pool.tile([P, KC, CHUNK], FP8, tag="x8")
                        x8_tiles[(c, i + 1)] = xi8
                    # ---- gating: g[e, b], then w = softmax over e ----
                    gps = psg.tile([E, CHUNK], F32, tag="g")
                    for kc in range(KC):
                        nc.tensor.matmul(
                            gps,
                            wg_sb[:, kc, :],
                            src[:, kc, :],
                            start=(kc == 0),
                            stop=(kc == KC - 1),
                        )
                    expg = spool.tile([E, CHUNK], BF16, tag="expg")
                    nc.scalar.activation(expg, gps, AF.Exp)
                    sums = psg.tile([1, CHUNK], F32, tag="g")
                    nc.tensor.matmul(
                        sums, sel_sb[:, MC, 0:1], expg, start=True, stop=True
                    )
                    rfast = spool.tile([1, CHUNK], F32, tag="rfast")
                    nc.vector.reciprocal_approx_fast(rfast, sums)
                    wps = psg.tile([E, CHUNK], F32, tag="g")
                    nc.tensor.matmul(wps, ones4, rfast, start=True, stop=True)
                    wsb = spool.tile([E, CHUNK], BF16, tag="wsb")
                    nc.vector.tensor_tensor(wsb, expg, wps, OP.mult)
                    # ---- V stage: h1 = tanh(V8flat.T @ xi8 / VSCALE) ----
                    h1s = []
                    for mc in range(MC):
                        vps = psmm.tile([P, CHUNK], F32, tag="mm")
                        for kk in range(KC // 2):
                            nc.tensor.matmul(
                                vps,
                                v_sb[:, i, mc, 2 * kk : 2 * kk + 2, :],
                                src8[:, 2 * kk : 2 * kk + 2, :],
                                start=(kk == 0),
                                stop=(kk == KC // 2 - 1),
                                perf_mode=PM.DoubleRow,
                            )
                        h1 = h1pool.tile([P, CHUNK], BF16, tag="h1")
                        nc.scalar.activation(h1, vps, AF.Tanh, scale=1.0 / VSCALE)
                        h1s.append(h1)
                    # ---- C stage (block-diag expert pairs) + gate scale ----
                    y8 = ypool.tile([P, MC, CHUNK], FP8, tag="y")
                    for mc in range(MC):
                        cps = psmm.tile([P, CHUNK], F32, tag="mm")
                        nc.tensor.matmul(
                            cps, c_sb[:, i, mc, :], h1s[mc], start=True, stop=True
                        )
                        wbp = pswb.tile([P, CHUNK], F32, tag="wb")
                        nc.tensor.matmul(
                            wbp, sel_sb[:, mc, :], wsb, start=True, stop=True
                        )
                        h2 = h2pool.tile([P, CHUNK], BF16, tag="h2")
                        nc.scalar.activation(h2, cps, AF.Tanh)
                        nc.vector.tensor_tensor(y8[:, mc, :], h2, wbp, OP.mult)
                    # ---- U stage (fp8 DoubleRow) + fused combine ----
                    if bias_zero:
                        for d2 in range(KC // 2):
                            ups2 = psu.tile([P, 2, CHUNK], F32, tag="u")
                            for j in range(2):
                                dc = 2 * d2 + j
                                for m2 in range(MC // 2):
                                    nc.tensor.matmul(
                                        ups2[:, j, :],
                                        u_sb[
                                            :, i, 2 * m2 : 2 * m2 + 2,
                                            dc * P : (dc + 1) * P,
                                        ],
                                        y8[:, 2 * m2 : 2 * m2 + 2, :],
                                        start=(m2 == 0),
                                        stop=(m2 == MC // 2 - 1),
                                        perf_mode=PM.DoubleRow,
                                    )
                            ds = slice(2 * d2, 2 * d2 + 2)
                            tmp = tpool.tile([P, 2, CHUNK], BF16, tag="tmp")
                            nc.vector.tensor_tensor(
                                tmp, ups2, x0d[:, ds], OP.mult
                            )
                            eng = nc.gpsimd if d2 % 2 == 0 else nc.vector
                            eng.tensor_tensor(
                                xi[:, ds], tmp, src[:, ds], OP.add
                            )
                            if not last:
                                nc.scalar.copy(xi8[:, ds], xi[:, ds])
                            else:
                                nc.scalar.dma_start(outq[c, :, ds], xi[:, ds])
                    else:
                        for dc in range(KC):
                            ups = psu.tile([P, 2, CHUNK], F32, tag="u")
                            for m2 in range(MC // 2):
                                nc.tensor.matmul(
                                    ups[:, 0, :],
                                    u_sb[
                                        :, i, 2 * m2 : 2 * m2 + 2,
                                        dc * P : (dc + 1) * P,
                                    ],
                                    y8[:, 2 * m2 : 2 * m2 + 2, :],
                                    start=(m2 == 0),
                                    stop=(m2 == MC // 2 - 1),
                                    perf_mode=PM.DoubleRow,
                                )
                            tmp = tpool.tile([P, CHUNK], BF16, tag="tmp")
                            nc.vector.scalar_tensor_tensor(
                                tmp,
                                ups[:, 0, :],
                                b_sb[:, i, dc : dc + 1],
                                x0d[:, dc, :],
                                OP.add,
                                OP.mult,
                            )
                            eng = nc.gpsimd if dc % 2 == 0 else nc.vector
                            eng.tensor_tensor(
                                xi[:, dc, :], tmp, src[:, dc, :], OP.add
                            )
                            if not last:
                                nc.scalar.copy(xi8[:, dc, :], xi[:, dc, :])
                            else:
                                nc.scalar.dma_start(outq[c, :, dc], xi[:, dc, :])
    nc.compile()
    return nc


_CTX = {}


def _get_nc(bias_zero):
    key = ("nc", bias_zero)
    if key not in _CTX:
        _CTX[key] = _build(bias_zero)
    return _CTX[key]


def _prep_weights(U, V, C, Wg, b):
    f = np.float32
    U = np.asarray(U, dtype=f)
    V = np.asarray(V, dtype=f)
    C = np.asarray(C, dtype=f)
    Wg = np.asarray(Wg, dtype=f)
    b = np.asarray(b, dtype=f)
    # Vl[i, d, e*R+r] = V[i, e, d, r]; then partition-major
    # V8p[p, i, mc, kc, q] = VSCALE*Vl[i, kc*P+p, mc*P+q]
    Vl = V.transpose(0, 2, 1, 3).reshape(N_CROSS, KC, P, MC, P)
    V8p = np.ascontiguousarray((Vl.transpose(2, 0, 3, 1, 4) * VSCALE).astype(NPF8))
    # Ul[i, e*R+r, d] = U[i, e, d, r]; U8p[p, i, mc, d] = USCALE*Ul[i, mc*P+p, d]
    Ul = U.transpose(0, 1, 3, 2).reshape(N_CROSS, MC, P, D)
    U8p = np.ascontiguousarray((Ul.transpose(2, 0, 1, 3) * USCALE).astype(NPF8))
    # block-diagonal expert pairs for the C stage: Cp[p, i, m, q]
    Cb = np.zeros((N_CROSS, MC, P, P), dtype=f)
    for i in range(N_CROSS):
        for m in range(MC):
            Cb[i, m, :R, :R] = C[i, 2 * m]
            Cb[i, m, R:, R:] = C[i, 2 * m + 1]
    Cp = np.ascontiguousarray(Cb.transpose(2, 0, 1, 3).astype(NPBF))
    # Wgp[p, kc, e] = Wg[e, kc*P+p]
    Wgp = np.ascontiguousarray(
        Wg.reshape(E, KC, P).transpose(2, 1, 0).astype(NPBF)
    )
    # bp[p, i, kc] = (USCALE*YSCALE) * b[i, kc*P+p]  (matches the scaled ups)
    bpa = np.ascontiguousarray(
        b.reshape(N_CROSS, KC, P).transpose(2, 0, 1) * (USCALE * YSCALE)
    )
    # selector planes for broadcasting gate weights over ranks + a ones plane
    sel = np.zeros((E, MC + 1, P), dtype=NPBF)
    for m in range(MC):
        for j in range(P):
            sel[2 * m + j // R, m, j] = 1.0
    sel[:, MC, :] = 1.0
    return dict(V8p=V8p, U8p=U8p, Cp=Cp, Wgp=Wgp, bp=bpa, selp=sel)


def kernel(x, U, V, C, Wg, b, _trace=False):
    bias_zero = bool(np.all(np.asarray(b) == 0.0))
    nc = _get_nc(bias_zero)
    w = _prep_weights(U, V, C, Wg, b)
    xf = np.asarray(x, dtype=np.float32)
    # xq[c, p, kc, j] = x[core*BC + c*CHUNK + j, kc*P + p]
    xs = xf.reshape(NCORES, NCHUNK, CHUNK, KC, P).transpose(0, 1, 4, 3, 2)
    in_maps = []
    for ci in range(NCORES):
        m = {
            "xq": np.ascontiguousarray(xs[ci].astype(NPBF)),
            "xqd": np.ascontiguousarray(
                (xs[ci] * (1.0 / (USCALE * YSCALE))).astype(NPBF)
            ),
            "xq8": np.ascontiguousarray(xs[ci].astype(NPF8)),
        }
        m.update(w)
        in_maps.append(m)
    res = run_bass_kernel_spmd(nc, in_maps, list(range(NCORES)), trace=_trace)
    kernel.last_result = res
    # outq[c, p, kc, j] -> out[core rows, d]
    outs = []
    for ci in range(NCORES):
        oq = np.asarray(res.results[ci]["outq"])
        outs.append(oq.transpose(0, 3, 2, 1).reshape(BC, D))
    out = np.concatenate(outs, axis=0)
    return np.ascontiguousarray(out.astype(np.float32))


# revision 18
# speedup vs baseline: 1.3926x; 1.0820x over previous
"""CrossNetMix (DCN-V2 mixture-of-low-rank-experts) Trainium2 kernel.

Strategy: data-parallel over batch across 8 cores (2048 rows each), with all
tensors kept feature-major on chip ([d, b] layout) so the contraction dim of
every matmul lands on SBUF partitions and no on-chip transposes are needed.
The host pre-shuffles every tensor into the exact [partition, ...] layout the
kernel reads, so each DMA moves long contiguous per-partition runs.

The U stage runs fp8e4 DoubleRow matmuls (2 weights per PE cell, half the
matmul count); V / C / gating stay bf16 — running V in fp8 too pushes the
chip into the P0 power state and the PE clock drops 2.4 -> 2.0 GHz, a net
loss. Scales keep everything in e4m3 range: U8 = 16*U and y = h2*(4w) so
ups = 64x the true value; the combine consumes a pre-divided x0/64 copy so
no descale op is needed.

The loop is layer-major over all 4 batch chunks so chunk c+1's matmuls fill
chunk c's end-of-layer dependency gap and the PE never idles long enough to
re-throttle. When b == 0 (the common case) the combine runs on dc-PAIRS
([128, 1024] DVE ops over two PSUM banks) to halve elementwise op count.

Per layer (fused), per chunk:
  g = xi @ Wg.T                  -> gating matmuls, M=8
  w = softmax(g)                 -> exp on ACT; partition-sum / broadcast via
                                    tiny ones/selector matmuls on the PE
  h1 = tanh(xi @ Vflat)          -> [er=512, b] feature-major
  h2 = tanh(blockdiag_C @ h1)    -> per-expert C folded into 128x128
                                    block-diagonal pairs (4 matmuls)
  y8 = h2 * 4w_broadcast (fp8)
  ups = U8.T @ y8 (DoubleRow)    -> [d, b], 64x scaled
  xi' = (x0/64) * (ups + 64b) + xi -> fused combine from PSUM into fresh tile
"""

import sys

import numpy as np

if "/opt/trn_rl_repo" not in sys.path:
    sys.path.insert(0, "/opt/trn_rl_repo")

import concourse.bass as bass
import concourse.bacc as bacc
import concourse.mybir as mybir
from concourse.tile import TileContext
from concourse.bass_utils import run_bass_kernel_spmd

import ml_dtypes

AF = mybir.ActivationFunctionType
OP = mybir.AluOpType
PM = mybir.MatmulPerfMode
F32 = mybir.dt.float32
BF16 = mybir.dt.bfloat16
FP8 = mybir.dt.float8e4
NPBF = ml_dtypes.bfloat16
NPF8 = ml_dtypes.float8_e4m3

N_CROSS = 3
E = 8            # experts
D = 1024         # feature dim
R = 64           # low rank
B = 16384        # full batch
NCORES = 8
BC = B // NCORES  # rows per core
CHUNK = 512       # batch tile (matmul free dim)
NCHUNK = BC // CHUNK
P = 128
KC = D // P       # d-chunks
ER = E * R        # 512
MC = ER // P      # (e,r)-chunks

USCALE = 16.0     # host-side U-weight scale (fp8 range centering)
YSCALE = 4.0      # gate-weight scale applied on-chip via the recip row
VSCALE = 16.0     # host-side V-weight scale (descaled inside the h1 tanh)


def _build(bias_zero):
    nc = bacc.Bacc(None)
    # every DRAM tensor is laid out partition-major on the host so DMAs
    # stream long contiguous runs per partition
    xq = nc.declare_dram_parameter("xq", [NCHUNK, P, KC, CHUNK], BF16, isOutput=False)
    xqd = nc.declare_dram_parameter("xqd", [NCHUNK, P, KC, CHUNK], BF16, isOutput=False)
    Vp = nc.declare_dram_parameter("Vp", [P, N_CROSS, MC, KC, P], BF16, isOutput=False)
    Cp = nc.declare_dram_parameter("Cp", [P, N_CROSS, MC, P], BF16, isOutput=False)
    U8p = nc.declare_dram_parameter("U8p", [P, N_CROSS, MC, D], FP8, isOutput=False)
    Wgp = nc.declare_dram_parameter("Wgp", [P, KC, E], BF16, isOutput=False)
    bp = nc.declare_dram_parameter("bp", [P, N_CROSS, KC], F32, isOutput=False)
    selp = nc.declare_dram_parameter("selp", [E, MC + 1, P], BF16, isOutput=False)
    outq = nc.declare_dram_parameter("outq", [NCHUNK, P, KC, CHUNK], BF16, isOutput=True)

    with TileContext(nc) as tc:
        with (
            tc.sbuf_pool(name="wpool", bufs=1) as wpool,
            tc.sbuf_pool(name="xpool", bufs=NCHUNK) as xpool,
            tc.sbuf_pool(name="xdpool", bufs=NCHUNK) as xdpool,
            tc.sbuf_pool(name="xipool", bufs=6) as xipool,
            tc.sbuf_pool(name="h1pool", bufs=6) as h1pool,
            tc.sbuf_pool(name="h2pool", bufs=3) as h2pool,
            tc.sbuf_pool(name="ypool", bufs=3) as ypool,
            tc.sbuf_pool(name="tpool", bufs=4) as tpool,
            tc.sbuf_pool(name="spool", bufs=2) as spool,
            tc.psum_pool(name="psmm", bufs=2) as psmm,
            tc.psum_pool(name="psu", bufs=2) as psu,
            tc.psum_pool(name="pswb", bufs=1) as pswb,
            tc.psum_pool(name="psg", bufs=1) as psg,
        ):
            v_sb = wpool.tile([P, N_CROSS, MC, KC, P], BF16)
            u_sb = wpool.tile([P, N_CROSS, MC, D], FP8)
            c_sb = wpool.tile([P, N_CROSS, MC, P], BF16)

            def load_x0(c, parts=2, pool=None, src_dram=None, tag="x0"):
                t = pool.tile([P, KC, CHUNK], BF16, tag=tag, name=f"{tag}_{c}")
                step = KC // parts
                for q in range(parts):
                    sl = slice(q * step, (q + 1) * step)
                    nc.sync.dma_start(t[:, sl], src_dram[c, :, sl])
                return t

            # critical-path-first DMA order: gate weights, then interleaved
            # chunk-0 x parts with layer-0 V parts; then everything else.
            wg_sb = wpool.tile([P, KC, E], BF16)
            nc.sync.dma_start(wg_sb, Wgp[:])
            ones4 = wpool.tile([1, E], F32)
            nc.vector.memset(ones4, YSCALE)
            x0_t = xpool.tile([P, KC, CHUNK], BF16, tag="x0", name="x0_0")
            for q in range(4):
                sl = slice(q * 2, (q + 1) * 2)
                nc.sync.dma_start(v_sb[:, 0, q], Vp[:, 0, q])
                nc.sync.dma_start(x0_t[:, sl], xq[0, :, sl])
            x0_tiles = [x0_t]
            sel_sb = wpool.tile([E, MC + 1, P], BF16)
            nc.sync.dma_start(sel_sb, selp[:])
            b_sb = wpool.tile([P, N_CROSS, KC], F32)
            nc.sync.dma_start(b_sb, bp[:])
            nc.sync.dma_start(c_sb[:, 0], Cp[:, 0])
            nc.sync.dma_start(u_sb[:, 0], U8p[:, 0])
            x0d_tiles = [load_x0(0, pool=xdpool, src_dram=xqd, tag="x0d")]
            for c in range(1, NCHUNK):
                x0_tiles.append(load_x0(c, pool=xpool, src_dram=xq))
                x0d_tiles.append(load_x0(c, pool=xdpool, src_dram=xqd, tag="x0d"))
            for i in range(1, N_CROSS):
                for mc in range(MC):
                    nc.sync.dma_start(v_sb[:, i, mc], Vp[:, i, mc])
                nc.sync.dma_start(c_sb[:, i], Cp[:, i])
                nc.sync.dma_start(u_sb[:, i], U8p[:, i])

            xi_tiles = {(c, 0): x0_tiles[c] for c in range(NCHUNK)}

            for i in range(N_CROSS):
                for c in range(NCHUNK):
                    x0d = x0d_tiles[c]
                    src = xi_tiles.pop((c, i))
                    xi = xipool.tile([P, KC, CHUNK], BF16, tag="xi")
                    xi_tiles[(c, i + 1)] = xi
                    last = i + 1 == N_CROSS
                    # ---- gating: g[e, b], then w = softmax over e ----
                    gps = psg.tile([E, CHUNK], F32, tag="g")
                    for kc in range(KC):
                        nc.tensor.matmul(
                            gps,
                            wg_sb[:, kc, :],
                            src[:, kc, :],
                            start=(kc == 0),
                            stop=(kc == KC - 1),
                        )
                    expg = spool.tile([E, CHUNK], BF16, tag="expg")
                    nc.scalar.activation(expg, gps, AF.Exp)
                    sums = psg.tile([1, CHUNK], F32, tag="g")
                    nc.tensor.matmul(
                        sums, sel_sb[:, MC, 0:1], expg, start=True, stop=True
                    )
                    rfast = spool.tile([1, CHUNK], F32, tag="rfast")
                    nc.vector.reciprocal_approx_fast(rfast, sums)
                    wps = psg.tile([E, CHUNK], F32, tag="g")
                    nc.tensor.matmul(wps, ones4, rfast, start=True, stop=True)
                    wsb = spool.tile([E, CHUNK], BF16, tag="wsb")
                    nc.vector.tensor_tensor(wsb, expg, wps, OP.mult)
                    # ---- V stage: h1 = tanh(Vflat.T @ xi) ----
                    h1s = []
                    for mc in range(MC):
                        vps = psmm.tile([P, CHUNK], F32, tag="mm")
                        for kc in range(KC):
                            nc.tensor.matmul(
                                vps,
                                v_sb[:, i, mc, kc, :],
                                src[:, kc, :],
                                start=(kc == 0),
                                stop=(kc == KC - 1),
                            )
                        h1 = h1pool.tile([P, CHUNK], BF16, tag="h1")
                        nc.scalar.activation(h1, vps, AF.Tanh)
                        h1s.append(h1)
                    # ---- C stage (block-diag expert pairs) + gate scale ----
                    y8 = ypool.tile([P, MC, CHUNK], FP8, tag="y")
                    for mc in range(MC):
                        cps = psmm.tile([P, CHUNK], F32, tag="mm")
                        nc.tensor.matmul(
                            cps, c_sb[:, i, mc, :], h1s[mc], start=True, stop=True
                        )
                        wbp = pswb.tile([P, CHUNK], F32, tag="wb")
                        nc.tensor.matmul(
                            wbp, sel_sb[:, mc, :], wsb, start=True, stop=True
                        )
                        h2 = h2pool.tile([P, CHUNK], BF16, tag="h2")
                        nc.scalar.activation(h2, cps, AF.Tanh)
                        nc.vector.tensor_tensor(y8[:, mc, :], h2, wbp, OP.mult)
                    # ---- U stage (fp8 DoubleRow) + fused combine ----
                    if bias_zero:
                        for d2 in range(KC // 2):
                            ups2 = psu.tile([P, 2, CHUNK], F32, tag="u")
                            for j in range(2):
                                dc = 2 * d2 + j
                                for m2 in range(MC // 2):
                                    nc.tensor.matmul(
                                        ups2[:, j, :],
                                        u_sb[
                                            :, i, 2 * m2 : 2 * m2 + 2,
                                            dc * P : (dc + 1) * P,
                                        ],
                                        y8[:, 2 * m2 : 2 * m2 + 2, :],
                                        start=(m2 == 0),
                                        stop=(m2 == MC // 2 - 1),
                                        perf_mode=PM.DoubleRow,
                                    )
                            ds = slice(2 * d2, 2 * d2 + 2)
                            tmp = tpool.tile([P, 2, CHUNK], BF16, tag="tmp")
                            nc.vector.tensor_tensor(
                                tmp, ups2, x0d[:, ds], OP.mult
                            )
                            nc.vector.tensor_tensor(
                                xi[:, ds], tmp, src[:, ds], OP.add
                            )
                            if last:
                                nc.scalar.dma_start(outq[c, :, ds], xi[:, ds])
                    else:
                        for dc in range(KC):
                            ups = psu.tile([P, 2, CHUNK], F32, tag="u")
                            for m2 in range(MC // 2):
                                nc.tensor.matmul(
                                    ups[:, 0, :],
                                    u_sb[
                                        :, i, 2 * m2 : 2 * m2 + 2,
                                        dc * P : (dc + 1) * P,
                                    ],
                                    y8[:, 2 * m2 : 2 * m2 + 2, :],
                                    start=(m2 == 0),
                                    stop=(m2 == MC // 2 - 1),
                                    perf_mode=PM.DoubleRow,
                                )
                            tmp = tpool.tile([P, CHUNK], BF16, tag="tmp")
                            nc.vector.scalar_tensor_tensor(
                                tmp,
                                ups[:, 0, :],
                                b_sb[:, i, dc : dc + 1],
                                x0d[:, dc, :],
                                OP.add,
                                OP.mult,
                            )
                            eng = nc.gpsimd if dc % 2 == 0 else nc.vector
                            eng.tensor_tensor(
                                xi[:, dc, :], tmp, src[:, dc, :], OP.add
                            )
                            if last:
                                nc.scalar.dma_start(outq[c, :, dc], xi[:, dc, :])
    nc.compile()
    return nc


_CTX = {}


def _get_nc(bias_zero):
    key = ("nc", bias_zero)
    if key not in _CTX:
        _CTX[key] = _build(bias_zero)
    return _CTX[key]


def _prep_weights(U, V, C, Wg, b):
    f = np.float32
    U = np.asarray(U, dtype=f)
    V = np.asarray(V, dtype=f)
    C = np.asarray(C, dtype=f)
    Wg = np.asarray(Wg, dtype=f)
    b = np.asarray(b, dtype=f)
    # Vl[i, d, e*R+r] = V[i, e, d, r]; then partition-major
    # Vp[p, i, mc, kc, q] = Vl[i, kc*P+p, mc*P+q]
    Vl = V.transpose(0, 2, 1, 3).reshape(N_CROSS, KC, P, MC, P)
    Vp = np.ascontiguousarray(Vl.transpose(2, 0, 3, 1, 4).astype(NPBF))
    # Ul[i, e*R+r, d] = U[i, e, d, r]; U8p[p, i, mc, d] = USCALE*Ul[i, mc*P+p, d]
    Ul = U.transpose(0, 1, 3, 2).reshape(N_CROSS, MC, P, D)
    U8p = np.ascontiguousarray((Ul.transpose(2, 0, 1, 3) * USCALE).astype(NPF8))
    # block-diagonal expert pairs for the C stage: Cp[p, i, m, q]
    Cb = np.zeros((N_CROSS, MC, P, P), dtype=f)
    for i in range(N_CROSS):
        for m in range(MC):
            Cb[i, m, :R, :R] = C[i, 2 * m]
            Cb[i, m, R:, R:] = C[i, 2 * m + 1]
    Cp = np.ascontiguousarray(Cb.transpose(2, 0, 1, 3).astype(NPBF))
    # Wgp[p, kc, e] = Wg[e, kc*P+p]
    Wgp = np.ascontiguousarray(
        Wg.reshape(E, KC, P).transpose(2, 1, 0).astype(NPBF)
    )
    # bp[p, i, kc] = (USCALE*YSCALE) * b[i, kc*P+p]  (matches the scaled ups)
    bpa = np.ascontiguousarray(
        b.reshape(N_CROSS, KC, P).transpose(2, 0, 1) * (USCALE * YSCALE)
    )
    # selector planes for broadcasting gate weights over ranks + a ones plane
    sel = np.zeros((E, MC + 1, P), dtype=NPBF)
    for m in range(MC):
        for j in range(P):
            sel[2 * m + j // R, m, j] = 1.0
    sel[:, MC, :] = 1.0
    return dict(Vp=Vp, U8p=U8p, Cp=Cp, Wgp=Wgp, bp=bpa, selp=sel)


def kernel(x, U, V, C, Wg, b, _trace=False):
    bias_zero = bool(np.all(np.asarray(b) == 0.0))
    nc = _get_nc(bias_zero)
    w = _prep_weights(U, V, C, Wg, b)
    xf = np.asarray(x, dtype=np.float32)
    # xq[c, p, kc, j] = x[core*BC + c*CHUNK + j, kc*P + p]
    xs = xf.reshape(NCORES, NCHUNK, CHUNK, KC, P).transpose(0, 1, 4, 3, 2)
    in_maps = []
    for ci in range(NCORES):
        m = {
            "xq": np.ascontiguousarray(xs[ci].astype(NPBF)),
            "xqd": np.ascontiguousarray(
                (xs[ci] * (1.0 / (USCALE * YSCALE))).astype(NPBF)
            ),
        }
        m.update(w)
        in_maps.append(m)
    res = run_bass_kernel_spmd(nc, in_maps, list(range(NCORES)), trace=_trace)
    kernel.last_result = res
    # outq[c, p, kc, j] -> out[core rows, d]
    outs = []
    for ci in range(NCORES):
        oq = np.asarray(res.results[ci]["outq"])
        outs.append(oq.transpose(0, 3, 2, 1).reshape(BC, D))
    out = np.concatenate(outs, axis=0)
    return np.ascontiguousarray(out.astype(np.float32))


# revision 19
# speedup vs baseline: 1.4412x; 1.0349x over previous
"""CrossNetMix (DCN-V2 mixture-of-low-rank-experts) Trainium2 kernel.

Strategy: data-parallel over batch across 8 cores (2048 rows each), with all
tensors kept feature-major on chip ([d, b] layout) so the contraction dim of
every matmul lands on SBUF partitions and no on-chip transposes are needed.
The host pre-shuffles every tensor into the exact [partition, ...] layout the
kernel reads, so each DMA moves long contiguous per-partition runs.

The U stage runs fp8e4 DoubleRow matmuls (2 weights per PE cell, half the
matmul count); V / C / gating stay bf16 — running V in fp8 too pushes the
chip into the P0 power state and the PE clock drops 2.4 -> 2.0 GHz, a net
loss. Scales keep everything in e4m3 range: U8 = 16*U and y = h2*(4w) so
ups = 64x the true value; the combine consumes a pre-divided x0/64 copy so
no descale op is needed.

The loop is layer-major over all 4 batch chunks so chunk c+1's matmuls fill
chunk c's end-of-layer dependency gap and the PE never idles long enough to
re-throttle. When b == 0 (the common case) the combine runs on dc-PAIRS
([128, 1024] DVE ops over two PSUM banks) to halve elementwise op count.

Per layer (fused), per chunk:
  g = xi @ Wg.T                  -> gating matmuls, M=8
  w = softmax(g)                 -> exp on ACT; partition-sum / broadcast via
                                    tiny ones/selector matmuls on the PE
  h1 = tanh(xi @ Vflat)          -> [er=512, b] feature-major
  h2 = tanh(blockdiag_C @ h1)    -> per-expert C folded into 128x128
                                    block-diagonal pairs (4 matmuls)
  y8 = h2 * 4w_broadcast (fp8)
  ups = U8.T @ y8 (DoubleRow)    -> [d, b], 64x scaled
  xi' = (x0/64) * (ups + 64b) + xi -> fused combine from PSUM into fresh tile
"""

import sys

import numpy as np

if "/opt/trn_rl_repo" not in sys.path:
    sys.path.insert(0, "/opt/trn_rl_repo")

import concourse.bass as bass
import concourse.bacc as bacc
import concourse.mybir as mybir
from concourse.tile import TileContext
from concourse.bass_utils import run_bass_kernel_spmd

import ml_dtypes

AF = mybir.ActivationFunctionType
OP = mybir.AluOpType
PM = mybir.MatmulPerfMode
F32 = mybir.dt.float32
BF16 = mybir.dt.bfloat16
FP8 = mybir.dt.float8e4
NPBF = ml_dtypes.bfloat16
NPF8 = ml_dtypes.float8_e4m3

N_CROSS = 3
E = 8            # experts
D = 1024         # feature dim
R = 64           # low rank
B = 16384        # full batch
NCORES = 8
BC = B // NCORES  # rows per core
CHUNK = 512       # batch tile (matmul free dim)
NCHUNK = BC // CHUNK
P = 128
KC = D // P       # d-chunks
ER = E * R        # 512
MC = ER // P      # (e,r)-chunks

USCALE = 16.0     # host-side U-weight scale (fp8 range centering)
YSCALE = 4.0      # gate-weight scale applied on-chip via the recip row
VSCALE = 16.0     # host-side V-weight scale (descaled inside the h1 tanh)


def _build(bias_zero):
    nc = bacc.Bacc(None)
    # every DRAM tensor is laid out partition-major on the host so DMAs
    # stream long contiguous runs per partition
    xq = nc.declare_dram_parameter("xq", [NCHUNK, P, KC, CHUNK], BF16, isOutput=False)
    xqd = nc.declare_dram_parameter("xqd", [NCHUNK, P, KC, CHUNK], BF16, isOutput=False)
    Vp = nc.declare_dram_parameter("Vp", [P, N_CROSS, MC, KC, P], BF16, isOutput=False)
    Cp = nc.declare_dram_parameter("Cp", [P, N_CROSS, MC, P], BF16, isOutput=False)
    U8p = nc.declare_dram_parameter("U8p", [P, N_CROSS, MC, D], FP8, isOutput=False)
    Wgp = nc.declare_dram_parameter("Wgp", [P, KC, E], BF16, isOutput=False)
    bp = nc.declare_dram_parameter("bp", [P, N_CROSS, KC], F32, isOutput=False)
    selp = nc.declare_dram_parameter("selp", [E, MC + 1, P], BF16, isOutput=False)
    outq = nc.declare_dram_parameter("outq", [NCHUNK, P, KC, CHUNK], BF16, isOutput=True)

    with TileContext(nc) as tc:
        with (
            tc.sbuf_pool(name="wpool", bufs=1) as wpool,
            tc.sbuf_pool(name="xpool", bufs=NCHUNK) as xpool,
            tc.sbuf_pool(name="xdpool", bufs=NCHUNK) as xdpool,
            tc.sbuf_pool(name="xipool", bufs=6) as xipool,
            tc.sbuf_pool(name="h1pool", bufs=6) as h1pool,
            tc.sbuf_pool(name="h2pool", bufs=3) as h2pool,
            tc.sbuf_pool(name="ypool", bufs=3) as ypool,
            tc.sbuf_pool(name="tpool", bufs=4) as tpool,
            tc.sbuf_pool(name="spool", bufs=2) as spool,
            tc.psum_pool(name="psmm", bufs=3) as psmm,
            tc.psum_pool(name="psu", bufs=2) as psu,
            tc.psum_pool(name="pswb", bufs=2) as pswb,
            tc.psum_pool(name="psg", bufs=1) as psg,
        ):
            v_sb = wpool.tile([P, N_CROSS, MC, KC, P], BF16)
            u_sb = wpool.tile([P, N_CROSS, MC, D], FP8)
            c_sb = wpool.tile([P, N_CROSS, MC, P], BF16)

            def load_x0(c, parts=2, pool=None, src_dram=None, tag="x0"):
                t = pool.tile([P, KC, CHUNK], BF16, tag=tag, name=f"{tag}_{c}")
                step = KC // parts
                for q in range(parts):
                    sl = slice(q * step, (q + 1) * step)
                    nc.sync.dma_start(t[:, sl], src_dram[c, :, sl])
                return t

            # critical-path-first DMA order: gate weights, then interleaved
            # chunk-0 x parts with layer-0 V parts; then everything else.
            wg_sb = wpool.tile([P, KC, E], BF16)
            nc.sync.dma_start(wg_sb, Wgp[:])
            ones4 = wpool.tile([1, E], F32)
            nc.vector.memset(ones4, YSCALE)
            x0_t = xpool.tile([P, KC, CHUNK], BF16, tag="x0", name="x0_0")
            for q in range(4):
                sl = slice(q * 2, (q + 1) * 2)
                nc.sync.dma_start(v_sb[:, 0, q], Vp[:, 0, q])
                nc.sync.dma_start(x0_t[:, sl], xq[0, :, sl])
            x0_tiles = [x0_t]
            sel_sb = wpool.tile([E, MC + 1, P], BF16)
            nc.sync.dma_start(sel_sb, selp[:])
            b_sb = wpool.tile([P, N_CROSS, KC], F32)
            nc.sync.dma_start(b_sb, bp[:])
            nc.sync.dma_start(c_sb[:, 0], Cp[:, 0])
            nc.sync.dma_start(u_sb[:, 0], U8p[:, 0])
            x0d_tiles = [load_x0(0, pool=xdpool, src_dram=xqd, tag="x0d")]
            for c in range(1, NCHUNK):
                x0_tiles.append(load_x0(c, pool=xpool, src_dram=xq))
                x0d_tiles.append(load_x0(c, pool=xdpool, src_dram=xqd, tag="x0d"))
            for i in range(1, N_CROSS):
                for mc in range(MC):
                    nc.sync.dma_start(v_sb[:, i, mc], Vp[:, i, mc])
                nc.sync.dma_start(c_sb[:, i], Cp[:, i])
                nc.sync.dma_start(u_sb[:, i], U8p[:, i])

            xi_tiles = {(c, 0): x0_tiles[c] for c in range(NCHUNK)}

            for i in range(N_CROSS):
                for c in range(NCHUNK):
                    x0d = x0d_tiles[c]
                    src = xi_tiles.pop((c, i))
                    xi = xipool.tile([P, KC, CHUNK], BF16, tag="xi")
                    xi_tiles[(c, i + 1)] = xi
                    last = i + 1 == N_CROSS
                    # ---- gating: g[e, b], then w = softmax over e ----
                    gps = psg.tile([E, CHUNK], F32, tag="g")
                    for kc in range(KC):
                        nc.tensor.matmul(
                            gps,
                            wg_sb[:, kc, :],
                            src[:, kc, :],
                            start=(kc == 0),
                            stop=(kc == KC - 1),
                        )
                    expg = spool.tile([E, CHUNK], BF16, tag="expg")
                    nc.scalar.activation(expg, gps, AF.Exp)
                    sums = psg.tile([1, CHUNK], F32, tag="g")
                    nc.tensor.matmul(
                        sums, sel_sb[:, MC, 0:1], expg, start=True, stop=True
                    )
                    rfast = spool.tile([1, CHUNK], F32, tag="rfast")
                    nc.vector.reciprocal_approx_fast(rfast, sums)
                    wps = psg.tile([E, CHUNK], F32, tag="g")
                    nc.tensor.matmul(wps, ones4, rfast, start=True, stop=True)
                    wsb = spool.tile([E, CHUNK], BF16, tag="wsb")
                    nc.vector.tensor_tensor(wsb, expg, wps, OP.mult)
                    # ---- V stage: h1 = tanh(Vflat.T @ xi) ----
                    h1s = []
                    for mc in range(MC):
                        vps = psmm.tile([P, CHUNK], F32, tag="mm")
                        for kc in range(KC):
                            nc.tensor.matmul(
                                vps,
                                v_sb[:, i, mc, kc, :],
                                src[:, kc, :],
                                start=(kc == 0),
                                stop=(kc == KC - 1),
                            )
                        h1 = h1pool.tile([P, CHUNK], BF16, tag="h1")
                        nc.scalar.activation(h1, vps, AF.Tanh)
                        h1s.append(h1)
                    # ---- C stage (block-diag expert pairs) + gate scale ----
                    y8 = ypool.tile([P, MC, CHUNK], FP8, tag="y")
                    for mc in range(MC):
                        cps = psmm.tile([P, CHUNK], F32, tag="mm")
                        nc.tensor.matmul(
                            cps, c_sb[:, i, mc, :], h1s[mc], start=True, stop=True
                        )
                        wbp = pswb.tile([P, CHUNK], F32, tag="wb")
                        nc.tensor.matmul(
                            wbp, sel_sb[:, mc, :], wsb, start=True, stop=True
                        )
                        h2 = h2pool.tile([P, CHUNK], BF16, tag="h2")
                        nc.scalar.activation(h2, cps, AF.Tanh)
                        nc.vector.tensor_tensor(y8[:, mc, :], h2, wbp, OP.mult)
                    # ---- U stage (fp8 DoubleRow) + fused combine ----
                    for dc in range(KC):
                        ups = psu.tile([P, CHUNK], F32, tag="u")
                        for m2 in range(MC // 2):
                            nc.tensor.matmul(
                                ups,
                                u_sb[
                                    :, i, 2 * m2 : 2 * m2 + 2,
                                    dc * P : (dc + 1) * P,
                                ],
                                y8[:, 2 * m2 : 2 * m2 + 2, :],
                                start=(m2 == 0),
                                stop=(m2 == MC // 2 - 1),
                                perf_mode=PM.DoubleRow,
                            )
                        tmp = tpool.tile([P, CHUNK], BF16, tag="tmp")
                        if bias_zero:
                            nc.vector.tensor_tensor(
                                tmp, ups, x0d[:, dc, :], OP.mult
                            )
                        else:
                            nc.vector.scalar_tensor_tensor(
                                tmp,
                                ups,
                                b_sb[:, i, dc : dc + 1],
                                x0d[:, dc, :],
                                OP.add,
                                OP.mult,
                            )
                        eng = nc.gpsimd if dc % 2 == 0 else nc.vector
                        eng.tensor_tensor(
                            xi[:, dc, :], tmp, src[:, dc, :], OP.add
                        )
                        if last:
                            nc.scalar.dma_start(outq[c, :, dc], xi[:, dc, :])
    nc.compile()
    return nc


_CTX = {}


def _get_nc(bias_zero):
    key = ("nc", bias_zero)
    if key not in _CTX:
        _CTX[key] = _build(bias_zero)
    return _CTX[key]


def _prep_weights(U, V, C, Wg, b):
    f = np.float32
    U = np.asarray(U, dtype=f)
    V = np.asarray(V, dtype=f)
    C = np.asarray(C, dtype=f)
    Wg = np.asarray(Wg, dtype=f)
    b = np.asarray(b, dtype=f)
    # Vl[i, d, e*R+r] = V[i, e, d, r]; then partition-major
    # Vp[p, i, mc, kc, q] = Vl[i, kc*P+p, mc*P+q]
    Vl = V.transpose(0, 2, 1, 3).reshape(N_CROSS, KC, P, MC, P)
    Vp = np.ascontiguousarray(Vl.transpose(2, 0, 3, 1, 4).astype(NPBF))
    # Ul[i, e*R+r, d] = U[i, e, d, r]; U8p[p, i, mc, d] = USCALE*Ul[i, mc*P+p, d]
    Ul = U.transpose(0, 1, 3, 2).reshape(N_CROSS, MC, P, D)
    U8p = np.ascontiguousarray((Ul.transpose(2, 0, 1, 3) * USCALE).astype(NPF8))
    # block-diagonal expert pairs for the C stage: Cp[p, i, m, q]
    Cb = np.zeros((N_CROSS, MC, P, P), dtype=f)
    for i in range(N_CROSS):
        for m in range(MC):
            Cb[i, m, :R, :R] = C[i, 2 * m]
            Cb[i, m, R:, R:] = C[i, 2 * m + 1]
    Cp = np.ascontiguousarray(Cb.transpose(2, 0, 1, 3).astype(NPBF))
    # Wgp[p, kc, e] = Wg[e, kc*P+p]
    Wgp = np.ascontiguousarray(
        Wg.reshape(E, KC, P).transpose(2, 1, 0).astype(NPBF)
    )
    # bp[p, i, kc] = (USCALE*YSCALE) * b[i, kc*P+p]  (matches the scaled ups)
    bpa = np.ascontiguousarray(
        b.reshape(N_CROSS, KC, P).transpose(2, 0, 1) * (USCALE * YSCALE)
    )
    # selector planes for broadcasting gate weights over ranks + a ones plane
    sel = np.zeros((E, MC + 1, P), dtype=NPBF)
    for m in range(MC):
        for j in range(P):
            sel[2 * m + j // R, m, j] = 1.0
    sel[:, MC, :] = 1.0
    return dict(Vp=Vp, U8p=U8p, Cp=Cp, Wgp=Wgp, bp=bpa, selp=sel)


def kernel(x, U, V, C, Wg, b, _trace=False):
    bias_zero = bool(np.all(np.asarray(b) == 0.0))
    nc = _get_nc(bias_zero)
    w = _prep_weights(U, V, C, Wg, b)
    xf = np.asarray(x, dtype=np.float32)
    # xq[c, p, kc, j] = x[core*BC + c*CHUNK + j, kc*P + p]
    xs = xf.reshape(NCORES, NCHUNK, CHUNK, KC, P).transpose(0, 1, 4, 3, 2)
    in_maps = []
    for ci in range(NCORES):
        m = {
            "xq": np.ascontiguousarray(xs[ci].astype(NPBF)),
            "xqd": np.ascontiguousarray(
                (xs[ci] * (1.0 / (USCALE * YSCALE))).astype(NPBF)
            ),
        }
        m.update(w)
        in_maps.append(m)
    res = run_bass_kernel_spmd(nc, in_maps, list(range(NCORES)), trace=_trace)
    kernel.last_result = res
    # outq[c, p, kc, j] -> out[core rows, d]
    outs = []
    for ci in range(NCORES):
        oq = np.asarray(res.results[ci]["outq"])
        outs.append(oq.transpose(0, 3, 2, 1).reshape(BC, D))
    out = np.concatenate(outs, axis=0)
    return np.ascontiguousarray(out.astype(np.float32))


# revision 20
# speedup vs baseline: 1.5000x; 1.0408x over previous
"""CrossNetMix (DCN-V2 mixture-of-low-rank-experts) Trainium2 kernel.

Strategy: data-parallel over batch across 8 cores (2048 rows each), with all
tensors kept feature-major on chip ([d, b] layout) so the contraction dim of
every matmul lands on SBUF partitions and no on-chip transposes are needed.
The host pre-shuffles every tensor into the exact [partition, ...] layout the
kernel reads, so each DMA moves long contiguous per-partition runs.

The U stage runs fp8e4 DoubleRow matmuls (2 weights per PE cell, half the
matmul count); V / C / gating stay bf16 — running V in fp8 too pushes the
chip into the P0 power state and the PE clock drops 2.4 -> 2.0 GHz, a net
loss. Scales keep everything in e4m3 range: U8 = 16*U and y = h2*(4w) so
ups = 64x the true value; the combine consumes a pre-divided x0/64 copy so
no descale op is needed.

The loop is layer-major over all 4 batch chunks so chunk c+1's matmuls fill
chunk c's end-of-layer dependency gap and the PE never idles long enough to
re-throttle. When b == 0 (the common case) the combine runs on dc-PAIRS
([128, 1024] DVE ops over two PSUM banks) to halve elementwise op count.

Per layer (fused), per chunk:
  g = xi @ Wg.T                  -> gating matmuls, M=8
  w = softmax(g)                 -> exp on ACT; partition-sum / broadcast via
                                    tiny ones/selector matmuls on the PE
  h1 = tanh(xi @ Vflat)          -> [er=512, b] feature-major
  h2 = tanh(blockdiag_C @ h1)    -> per-expert C folded into 128x128
                                    block-diagonal pairs (4 matmuls)
  y8 = h2 * 4w_broadcast (fp8)
  ups = U8.T @ y8 (DoubleRow)    -> [d, b], 64x scaled
  xi' = (x0/64) * (ups + 64b) + xi -> fused combine from PSUM into fresh tile
"""

import sys

import numpy as np

if "/opt/trn_rl_repo" not in sys.path:
    sys.path.insert(0, "/opt/trn_rl_repo")

import concourse.bass as bass
import concourse.bacc as bacc
import concourse.mybir as mybir
from concourse.tile import TileContext
from concourse.bass_utils import run_bass_kernel_spmd

import ml_dtypes

AF = mybir.ActivationFunctionType
OP = mybir.AluOpType
PM = mybir.MatmulPerfMode
F32 = mybir.dt.float32
BF16 = mybir.dt.bfloat16
FP8 = mybir.dt.float8e4
NPBF = ml_dtypes.bfloat16
NPF8 = ml_dtypes.float8_e4m3

N_CROSS = 3
E = 8            # experts
D = 1024         # feature dim
R = 64           # low rank
B = 16384        # full batch
NCORES = 8
BC = B // NCORES  # rows per core
CHUNK = 512       # batch tile (matmul free dim)
NCHUNK = BC // CHUNK
P = 128
KC = D // P       # d-chunks
ER = E * R        # 512
MC = ER // P      # (e,r)-chunks

USCALE = 16.0     # host-side U-weight scale (fp8 range centering)
YSCALE = 4.0      # gate-weight scale applied on-chip via the recip row
VSCALE = 16.0     # host-side V-weight scale (descaled inside the h1 tanh)


def _build(bias_zero):
    nc = bacc.Bacc(None)
    # every DRAM tensor is laid out partition-major on the host so DMAs
    # stream long contiguous runs per partition
    xq = nc.declare_dram_parameter("xq", [NCHUNK, P, KC, CHUNK], BF16, isOutput=False)
    xqd = nc.declare_dram_parameter("xqd", [NCHUNK, P, KC, CHUNK], BF16, isOutput=False)
    Vp = nc.declare_dram_parameter("Vp", [P, N_CROSS, MC, KC, P], BF16, isOutput=False)
    Cp = nc.declare_dram_parameter("Cp", [P, N_CROSS, MC, P], BF16, isOutput=False)
    U8p = nc.declare_dram_parameter("U8p", [P, N_CROSS, MC, D], FP8, isOutput=False)
    Wgp = nc.declare_dram_parameter("Wgp", [P, KC, E], BF16, isOutput=False)
    bp = nc.declare_dram_parameter("bp", [P, N_CROSS, KC], F32, isOutput=False)
    selp = nc.declare_dram_parameter("selp", [E, MC + 1, P], BF16, isOutput=False)
    outq = nc.declare_dram_parameter("outq", [NCHUNK, P, KC, CHUNK], BF16, isOutput=True)

    with TileContext(nc) as tc:
        with (
            tc.sbuf_pool(name="wpool", bufs=1) as wpool,
            tc.sbuf_pool(name="xpool", bufs=NCHUNK) as xpool,
            tc.sbuf_pool(name="xdpool", bufs=NCHUNK) as xdpool,
            tc.sbuf_pool(name="xipool", bufs=6) as xipool,
            tc.sbuf_pool(name="h1pool", bufs=6) as h1pool,
            tc.sbuf_pool(name="h2pool", bufs=3) as h2pool,
            tc.sbuf_pool(name="ypool", bufs=3) as ypool,
            tc.sbuf_pool(name="tpool", bufs=4) as tpool,
            tc.sbuf_pool(name="spool", bufs=2) as spool,
            tc.psum_pool(name="psmm", bufs=3) as psmm,
            tc.psum_pool(name="psu", bufs=2) as psu,
            tc.psum_pool(name="pswb", bufs=2) as pswb,
            tc.psum_pool(name="psg", bufs=1) as psg,
        ):
            v_sb = wpool.tile([P, N_CROSS, MC, KC, P], BF16)
            u_sb = wpool.tile([P, N_CROSS, MC, D], FP8)
            c_sb = wpool.tile([P, N_CROSS, MC, P], BF16)

            def load_x0(c, parts=2, pool=None, src_dram=None, tag="x0"):
                t = pool.tile([P, KC, CHUNK], BF16, tag=tag, name=f"{tag}_{c}")
                step = KC // parts
                for q in range(parts):
                    sl = slice(q * step, (q + 1) * step)
                    nc.sync.dma_start(t[:, sl], src_dram[c, :, sl])
                return t

            # critical-path-first DMA order: gate weights, then interleaved
            # chunk-0 x parts with layer-0 V parts; then everything else.
            wg_sb = wpool.tile([P, KC, E], BF16)
            nc.sync.dma_start(wg_sb, Wgp[:])
            x0_t = xpool.tile([P, KC, CHUNK], BF16, tag="x0", name="x0_0")
            for q in range(4):
                sl = slice(q * 2, (q + 1) * 2)
                nc.sync.dma_start(v_sb[:, 0, q], Vp[:, 0, q])
                nc.sync.dma_start(x0_t[:, sl], xq[0, :, sl])
            x0_tiles = [x0_t]
            sel_sb = wpool.tile([E, MC + 1, P], BF16)
            nc.sync.dma_start(sel_sb, selp[:])
            b_sb = wpool.tile([P, N_CROSS, KC], F32)
            nc.sync.dma_start(b_sb, bp[:])
            nc.sync.dma_start(c_sb[:, 0], Cp[:, 0])
            nc.sync.dma_start(u_sb[:, 0], U8p[:, 0])
            x0d_tiles = [load_x0(0, pool=xdpool, src_dram=xqd, tag="x0d")]
            for c in range(1, NCHUNK):
                x0_tiles.append(load_x0(c, pool=xpool, src_dram=xq))
                x0d_tiles.append(load_x0(c, pool=xdpool, src_dram=xqd, tag="x0d"))
            for i in range(1, N_CROSS):
                for mc in range(MC):
                    nc.sync.dma_start(v_sb[:, i, mc], Vp[:, i, mc])
                nc.sync.dma_start(c_sb[:, i], Cp[:, i])
                nc.sync.dma_start(u_sb[:, i], U8p[:, i])

            xi_tiles = {(c, 0): x0_tiles[c] for c in range(NCHUNK)}

            for i in range(N_CROSS):
                for c in range(NCHUNK):
                    x0d = x0d_tiles[c]
                    src = xi_tiles.pop((c, i))
                    xi = xipool.tile([P, KC, CHUNK], BF16, tag="xi")
                    xi_tiles[(c, i + 1)] = xi
                    last = i + 1 == N_CROSS
                    # ---- gating: g[e, b], then w = softmax over e ----
                    gps = psg.tile([E, CHUNK], F32, tag="g")
                    for kc in range(KC):
                        nc.tensor.matmul(
                            gps,
                            wg_sb[:, kc, :],
                            src[:, kc, :],
                            start=(kc == 0),
                            stop=(kc == KC - 1),
                        )
                    expg = spool.tile([E, CHUNK], BF16, tag="expg")
                    nc.scalar.activation(expg, gps, AF.Exp)
                    sums = psg.tile([1, CHUNK], F32, tag="g")
                    nc.tensor.matmul(
                        sums, sel_sb[:, MC, 0:1], expg, start=True, stop=True
                    )
                    rfast = spool.tile([1, CHUNK], F32, tag="rfast")
                    nc.vector.reciprocal_approx_fast(rfast, sums)
                    rrow = spool.tile([1, CHUNK], BF16, tag="rrow")
                    nc.vector.tensor_scalar_mul(rrow, rfast, YSCALE)
                    wps = psg.tile([E, CHUNK], F32, tag="g")
                    nc.tensor.matmul(
                        wps, sel_sb[0:1, MC, 0:E], rrow, start=True, stop=True
                    )
                    wsb = spool.tile([E, CHUNK], BF16, tag="wsb")
                    nc.vector.tensor_tensor(wsb, expg, wps, OP.mult)
                    # ---- V stage: h1 = tanh(Vflat.T @ xi) ----
                    h1s = []
                    for mc in range(MC):
                        vps = psmm.tile([P, CHUNK], F32, tag="mm")
                        for kc in range(KC):
                            nc.tensor.matmul(
                                vps,
                                v_sb[:, i, mc, kc, :],
                                src[:, kc, :],
                                start=(kc == 0),
                                stop=(kc == KC - 1),
                            )
                        h1 = h1pool.tile([P, CHUNK], BF16, tag="h1")
                        nc.scalar.activation(h1, vps, AF.Tanh)
                        h1s.append(h1)
                    # ---- C stage (block-diag expert pairs) + gate scale ----
                    y8 = ypool.tile([P, MC, CHUNK], FP8, tag="y")
                    for mc in range(MC):
                        cps = psmm.tile([P, CHUNK], F32, tag="mm")
                        nc.tensor.matmul(
                            cps, c_sb[:, i, mc, :], h1s[mc], start=True, stop=True
                        )
                        wbp = pswb.tile([P, CHUNK], F32, tag="wb")
                        nc.tensor.matmul(
                            wbp, sel_sb[:, mc, :], wsb, start=True, stop=True
                        )
                        h2 = h2pool.tile([P, CHUNK], BF16, tag="h2")
                        nc.scalar.activation(h2, cps, AF.Tanh)
                        nc.vector.tensor_tensor(y8[:, mc, :], h2, wbp, OP.mult)
                    # ---- U stage (fp8 DoubleRow) + fused combine ----
                    for dc in range(KC):
                        ups = psu.tile([P, CHUNK], F32, tag="u")
                        for m2 in range(MC // 2):
                            nc.tensor.matmul(
                                ups,
                                u_sb[
                                    :, i, 2 * m2 : 2 * m2 + 2,
                                    dc * P : (dc + 1) * P,
                                ],
                                y8[:, 2 * m2 : 2 * m2 + 2, :],
                                start=(m2 == 0),
                                stop=(m2 == MC // 2 - 1),
                                perf_mode=PM.DoubleRow,
                            )
                        tmp = tpool.tile([P, CHUNK], BF16, tag="tmp")
                        if bias_zero:
                            nc.vector.tensor_tensor(
                                tmp, ups, x0d[:, dc, :], OP.mult
                            )
                        else:
                            nc.vector.scalar_tensor_tensor(
                                tmp,
                                ups,
                                b_sb[:, i, dc : dc + 1],
                                x0d[:, dc, :],
                                OP.add,
                                OP.mult,
                            )
                        eng = nc.gpsimd if dc % 2 == 0 else nc.vector
                        eng.tensor_tensor(
                            xi[:, dc, :], tmp, src[:, dc, :], OP.add
                        )
                        if last:
                            nc.scalar.dma_start(outq[c, :, dc], xi[:, dc, :])
    nc.compile()
    return nc


_CTX = {}


def _get_nc(bias_zero):
    key = ("nc", bias_zero)
    if key not in _CTX:
        _CTX[key] = _build(bias_zero)
    return _CTX[key]


def _prep_weights(U, V, C, Wg, b):
    f = np.float32
    U = np.asarray(U, dtype=f)
    V = np.asarray(V, dtype=f)
    C = np.asarray(C, dtype=f)
    Wg = np.asarray(Wg, dtype=f)
    b = np.asarray(b, dtype=f)
    # Vl[i, d, e*R+r] = V[i, e, d, r]; then partition-major
    # Vp[p, i, mc, kc, q] = Vl[i, kc*P+p, mc*P+q]
    Vl = V.transpose(0, 2, 1, 3).reshape(N_CROSS, KC, P, MC, P)
    Vp = np.ascontiguousarray(Vl.transpose(2, 0, 3, 1, 4).astype(NPBF))
    # Ul[i, e*R+r, d] = U[i, e, d, r]; U8p[p, i, mc, d] = USCALE*Ul[i, mc*P+p, d]
    Ul = U.transpose(0, 1, 3, 2).reshape(N_CROSS, MC, P, D)
    U8p = np.ascontiguousarray((Ul.transpose(2, 0, 1, 3) * USCALE).astype(NPF8))
    # block-diagonal expert pairs for the C stage: Cp[p, i, m, q]
    Cb = np.zeros((N_CROSS, MC, P, P), dtype=f)
    for i in range(N_CROSS):
        for m in range(MC):
            Cb[i, m, :R, :R] = C[i, 2 * m]
            Cb[i, m, R:, R:] = C[i, 2 * m + 1]
    Cp = np.ascontiguousarray(Cb.transpose(2, 0, 1, 3).astype(NPBF))
    # Wgp[p, kc, e] = Wg[e, kc*P+p]
    Wgp = np.ascontiguousarray(
        Wg.reshape(E, KC, P).transpose(2, 1, 0).astype(NPBF)
    )
    # bp[p, i, kc] = (USCALE*YSCALE) * b[i, kc*P+p]  (matches the scaled ups)
    bpa = np.ascontiguousarray(
        b.reshape(N_CROSS, KC, P).transpose(2, 0, 1) * (USCALE * YSCALE)
    )
    # selector planes for broadcasting gate weights over ranks + a ones plane
    sel = np.zeros((E, MC + 1, P), dtype=NPBF)
    for m in range(MC):
        for j in range(P):
            sel[2 * m + j // R, m, j] = 1.0
    sel[:, MC, :] = 1.0
    return dict(Vp=Vp, U8p=U8p, Cp=Cp, Wgp=Wgp, bp=bpa, selp=sel)


def kernel(x, U, V, C, Wg, b, _trace=False):
    bias_zero = bool(np.all(np.asarray(b) == 0.0))
    nc = _get_nc(bias_zero)
    w = _prep_weights(U, V, C, Wg, b)
    xf = np.asarray(x, dtype=np.float32)
    # xq[c, p, kc, j] = x[core*BC + c*CHUNK + j, kc*P + p]
    xs = xf.reshape(NCORES, NCHUNK, CHUNK, KC, P).transpose(0, 1, 4, 3, 2)
    in_maps = []
    for ci in range(NCORES):
        m = {
            "xq": np.ascontiguousarray(xs[ci].astype(NPBF)),
            "xqd": np.ascontiguousarray(
                (xs[ci] * (1.0 / (USCALE * YSCALE))).astype(NPBF)
            ),
        }
        m.update(w)
        in_maps.append(m)
    res = run_bass_kernel_spmd(nc, in_maps, list(range(NCORES)), trace=_trace)
    kernel.last_result = res
    # outq[c, p, kc, j] -> out[core rows, d]
    outs = []
    for ci in range(NCORES):
        oq = np.asarray(res.results[ci]["outq"])
        outs.append(oq.transpose(0, 3, 2, 1).reshape(BC, D))
    out = np.concatenate(outs, axis=0)
    return np.ascontiguousarray(out.astype(np.float32))
